# revision 7
# baseline (speedup 1.0000x reference)
"""MultiHeadGraphAttention Trainium2 kernel (pipelined v2).

Data-parallel over batch: core b computes batch element b (B=8, 8 cores).

Per-core math (one batch element, N=2048 nodes, U=256 units, H=8 heads, d=32):
  Q = x Wq, K = x Wk, V = x Wv
  sT[k,q]  = sum_d KT[d,k] QT[d,q]           (scores, transposed layout)
  e        = exp(sT/sqrt(d)) * adjT          (masked exp)
  ctxT[d,q] = sum_k V[k,d] e[k,q] ; Z[q] = sum_k e[k,q]
  out      = (ctxT/Z).T @ Wo + bo

Structure: the [N,N] score plane is processed in (qc, kb) blocks of
[128 keys x 512 queries], 4 head-PAIR PSUM tiles of [128, 2x512] each
(2 banks, double-buffered) so ACT exp / DVE mask / PE matmuls pipeline
across pairs and iterations.  Per-pair mode:
  'A': ACT exp (PSUM->SBUF bf16) then DVE mask-multiply (bf16 2x)
  'P': PE adds 170*mask into PSUM (diag matmul), ACT does exp(scale*s-30)
       -> masked entries become exp(s-30) ~= 1e-13; no DVE pass at all
  'S': DVE-only Schraudolph: i16 = A*s + B (1x from PSUM), bitcast to
       bf16 ~= exp(scale*s), then mask-multiply (2x)
Host pre-transposes x/adj and converts everything to bf16.
"""

import sys

for p in ("/opt/trn_rl_repo",):
    if p not in sys.path:
        sys.path.insert(0, p)

from contextlib import ExitStack

import numpy as np
import ml_dtypes

import concourse.bass as bass
import concourse.mybir as mybir
import concourse.tile as tile
from concourse import bacc
from concourse.bass_utils import run_bass_kernel_spmd

B, N, U, H, D = 8, 2048, 256, 8, 32
NB = N // 128          # 16 key blocks of 128
QC = 4                 # q chunks
QW = N // QC           # 512 q per chunk
SCALE = 1.0 / np.sqrt(np.float32(D))
MASK_GAIN = 170.0      # bf16-exact; PE adds 170*m to raw scores ('P' mode)
MASK_BIAS = -(MASK_GAIN * float(SCALE))   # ACT bias cancels it where m=1
# Schraudolph bf16: bits(bf16(2^x)) ~= round(128*(x+127-0.04368))
SCH_A = 128.0 * float(SCALE) * 1.4426950408889634
SCH_B = (127.0 - 0.04368) * 128.0

f32 = mybir.dt.float32
bf16 = mybir.dt.bfloat16
i16 = mybir.dt.int16
EXP = mybir.ActivationFunctionType.Exp
MULT = mybir.AluOpType.mult
ADD = mybir.AluOpType.add


def pair_mode(qc, kb, p):
    """Which engine path computes masked-exp for head pair p of (qc,kb).

    Rotating split: one pair per (qc,kb) goes to the DVE-Schraudolph path
    (rotates across pairs so the approx error spreads over heads), the
    other three use the PE-additive-mask + ACT-exp path (no DVE at all).
    """
    return "S" if p == kb % 4 else "P"


def build_program():
    nc = bacc.Bacc("TRN2", target_bir_lowering=False, debug=False,
                   enable_asserts=False, num_devices=B)

    xT_d = nc.dram_tensor("xT", [U, N], bf16, kind="ExternalInput").ap()
    adjT_d = nc.dram_tensor("adjT", [N, N], bf16, kind="ExternalInput").ap()
    wq_d = nc.dram_tensor("Wq", [U, U], bf16, kind="ExternalInput").ap()
    wk_d = nc.dram_tensor("Wk", [U, U], bf16, kind="ExternalInput").ap()
    wv_d = nc.dram_tensor("Wv", [U, U], bf16, kind="ExternalInput").ap()
    wo_d = nc.dram_tensor("Wo", [U, U], bf16, kind="ExternalInput").ap()
    bo_d = nc.dram_tensor("bo", [U], f32, kind="ExternalInput").ap()
    diag_d = nc.dram_tensor("diag", [128, 128], bf16, kind="ExternalInput").ap()
    out_d = nc.dram_tensor("out", [N, U], f32, kind="ExternalOutput").ap()

    with tile.TileContext(nc) as tc:
        with ExitStack() as ctx:
            kernel_body(ctx, tc, xT_d, adjT_d, wq_d, wk_d, wv_d, wo_d,
                        bo_d, diag_d, out_d)
    nc.compile()
    return nc


def kernel_body(ctx, tc, xT_d, adjT_d, wq_d, wk_d, wv_d, wo_d, bo_d,
                diag_d, out_d):
    nc = tc.nc
    persist = ctx.enter_context(tc.tile_pool(name="persist", bufs=1))
    stage = ctx.enter_context(tc.tile_pool(name="stage", bufs=2))
    epool = ctx.enter_context(tc.tile_pool(name="epool", bufs=6))
    espool = ctx.enter_context(tc.tile_pool(name="espool", bufs=2))
    spool = ctx.enter_context(tc.tile_pool(name="spool", bufs=2, space="PSUM"))
    cpool = ctx.enter_context(tc.tile_pool(name="cpool", bufs=2, space="PSUM"))
    zpool = ctx.enter_context(tc.tile_pool(name="zpool", bufs=2, space="PSUM"))

    # ---- persistent SBUF tensors -------------------------------------------
    # mask, bf16 0/1, block kb at cols kb*N (loaded first: feeds everything)
    m_sb = persist.tile([128, NB * N], bf16, tag="m")
    for kb in range(NB):
        nc.sync.dma_start(m_sb[:, kb * N:(kb + 1) * N],
                          adjT_d[kb * 128:(kb + 1) * 128, :])
    # projections, transposed: chunk g holds heads 4g..4g+3 (head h at
    # partitions 32*(h%4) .. +32)
    qT = [persist.tile([128, N], bf16, tag=f"qT{c}", name=f"qT{c}") for c in range(2)]
    kT = [persist.tile([128, N], bf16, tag=f"kT{c}", name=f"kT{c}") for c in range(2)]
    v_sb = persist.tile([128, NB * U], bf16, tag="v")
    # weights: [128, 2*256] bf16, feature chunk c at cols c*U
    w_sb = {}
    for nm, dram in (("wq", wq_d), ("wk", wk_d), ("wv", wv_d), ("wo", wo_d)):
        w_sb[nm] = persist.tile([128, 2 * U], bf16, tag=nm, name=nm)
        for c in range(2):
            nc.sync.dma_start(w_sb[nm][:, c * U:(c + 1) * U],
                              dram[c * 128:(c + 1) * 128, :])
    bo_sb = persist.tile([1, U], f32, tag="bo")
    nc.sync.dma_start(bo_sb[:], bo_d.rearrange("(o n) -> o n", o=1))
    diag_sb = persist.tile([128, 128], bf16, tag="diag")
    nc.sync.dma_start(diag_sb[:], diag_d)
    ones_bf = persist.tile([128, 1], bf16, tag="ones_bf")
    nc.vector.memset(ones_bf[:], 1.0)
    mbias = persist.tile([128, 1], f32, tag="mbias")
    nc.vector.memset(mbias[:], float(MASK_BIAS))
    ones_f = persist.tile([1, 128], f32, tag="ones_f")
    nc.vector.memset(ones_f[:], 1.0)
    # normalized context (transposed) bf16; final output rows f32
    ctxn = [persist.tile([128, N], bf16, tag=f"ctxn{c}", name=f"ctxn{c}") for c in range(2)]
    out_sb = persist.tile([128, NB * U], f32, tag="out_sb")
    # per-qc denominator scratch (full tiles; rows 32j hold head (g,j))
    zrec = [persist.tile([128, QW], f32, tag=f"zrec{g}", name=f"zrec{g}") for g in range(2)]
    zbs = [persist.tile([128, QW], f32, tag=f"zbs{g}", name=f"zbs{g}") for g in range(2)]
    zinv = [persist.tile([128, QW], f32, tag=f"zinv{g}", name=f"zinv{g}") for g in range(2)]

    # ---- projections (bf16 operands) ---------------------------------------
    xT = [stage.tile([128, N], bf16, tag="stage", name=f"xT{c}") for c in range(2)]
    for c in range(2):
        nc.sync.dma_start(xT[c][:], xT_d[c * 128:(c + 1) * 128, :])

    # Q/K: per dst chunk g, produce [128, N] via 4 x [128,512] PSUM pieces
    for w, dst in (("wq", qT), ("wk", kT)):
        for g in range(2):
            for nn in range(2):       # 1024-wide double-slices of N tokens
                ps = spool.tile([128, 2 * QW], f32, tag="s", name="projps")
                for half in range(2):
                    sl = slice(half * QW, (half + 1) * QW)
                    tok = slice(nn * 2 * QW + half * QW,
                                nn * 2 * QW + (half + 1) * QW)
                    for kc in range(2):
                        nc.tensor.matmul(
                            ps[:, sl],
                            w_sb[w][:, (kc * 2 + g) * 128:(kc * 2 + g + 1) * 128],
                            xT[kc][:, tok],
                            start=(kc == 0), stop=(kc == 1))
                nc.scalar.copy(dst[g][:, nn * 2 * QW:(nn + 1) * 2 * QW], ps[:])
    # V = x @ Wv, natural layout, bf16
    for kb in range(NB):
        ps = zpool.tile([128, U], f32, tag="z", name="vps")
        for kc in range(2):
            nc.tensor.matmul(
                ps[:],
                xT[kc][:, kb * 128:(kb + 1) * 128],
                w_sb["wv"][:, kc * U:(kc + 1) * U],
                start=(kc == 0), stop=(kc == 1))
        nc.vector.tensor_copy(v_sb[:, kb * U:(kb + 1) * U], ps[:])

    # ---- main attention loop, one-step PE software pipeline -----------------
    # pair p covers heads (2p, 2p+1): g = p//2, row/col groups j0=2*(p%2), j0+1
    cps = {}
    zps = {}

    def emit_scores(qc, kb):
        qs = qc * QW
        tiles = []
        for p in range(4):
            g, j0 = p // 2, 2 * (p % 2)
            mode = pair_mode(qc, kb, p)
            sps = spool.tile([128, 2 * QW], f32, tag="s", name=f"sps{qc}_{kb}_{p}")
            for jj in range(2):
                j = j0 + jj
                nc.tensor.matmul(
                    sps[:, jj * QW:(jj + 1) * QW],
                    kT[g][32 * j:32 * (j + 1), kb * 128:(kb + 1) * 128],
                    qT[g][32 * j:32 * (j + 1), qs:qs + QW],
                    start=True, stop=(mode != "P"),
                    tile_position=(32 * j, 0))
            if mode == "P":
                # accumulate 170*m into both head slices, full-array matmul
                me = m_sb[:, kb * N + qs:kb * N + qs + QW]
                for jj in range(2):
                    nc.tensor.matmul(
                        sps[:, jj * QW:(jj + 1) * QW],
                        diag_sb[:], me,
                        start=False, stop=True, skip_group_check=True)
            tiles.append((sps, mode))
        return tiles

    def emit_expmask_pvz(qc, kb, tiles):
        qs = qc * QW
        for p in range(4):
            g, j0 = p // 2, 2 * (p % 2)
            sps, mode = tiles[p]
            e = epool.tile([128, 2 * QW], bf16, tag="e", name=f"e{qc}_{kb}_{p}")
            me = m_sb[:, kb * N + qs:kb * N + qs + QW]
            if mode == "A":
                nc.scalar.activation(e[:], sps[:], EXP, scale=float(SCALE))
                nc.vector.tensor_tensor(
                    e.rearrange("p (j q) -> p j q", j=2),
                    e.rearrange("p (j q) -> p j q", j=2),
                    me.unsqueeze(1).broadcast_to([128, 2, QW]), MULT)
            elif mode == "P":
                nc.scalar.activation(e[:], sps[:], EXP, scale=float(SCALE),
                                     bias=mbias[:])
            else:  # "S"
                es = espool.tile([128, 2 * QW], i16, tag="es",
                                 name=f"es{qc}_{kb}_{p}")
                nc.vector.tensor_scalar(es[:], sps[:], float(SCH_A),
                                        float(SCH_B), MULT, ADD)
                nc.vector.tensor_tensor(
                    e.rearrange("p (j q) -> p j q", j=2),
                    es.bitcast(bf16).rearrange("p (j q) -> p j q", j=2),
                    me.unsqueeze(1).broadcast_to([128, 2, QW]), MULT)
            for jj in range(2):
                j = j0 + jj
                ej = e[:, jj * QW:(jj + 1) * QW]
                h = 4 * g + j
                nc.tensor.matmul(
                    cps[qc, g][32 * j:32 * (j + 1), :],
                    v_sb[:, kb * U + h * D:kb * U + (h + 1) * D],
                    ej, start=(kb == 0), stop=(kb == NB - 1),
                    tile_position=(0, 32 * j))
                nc.tensor.matmul(
                    zps[qc, g][32 * j:32 * j + 1, :],
                    ones_bf[:], ej,
                    start=(kb == 0), stop=(kb == NB - 1),
                    tile_position=(0, 32 * j))

    def emit_finish_qc(qc):
        qs = qc * QW
        for g in range(2):
            nc.vector.tensor_copy(zrec[g][:], zps[qc, g][:])
            for j in range(4):
                nc.sync.dma_start(
                    zbs[g][32 * j:32 * (j + 1), :],
                    zrec[g][32 * j:32 * j + 1, :]
                    .unsqueeze(1).broadcast_to([1, 32, QW]))
            nc.vector.reciprocal_approx_fast(zinv[g][:], zbs[g][:])
            nc.vector.tensor_tensor(ctxn[g][:, qs:qs + QW], cps[qc, g][:],
                                    zinv[g][:], MULT)
        for qb in range(qc * QC, (qc + 1) * QC):
            ops = zpool.tile([128, U], f32, tag="z", name=f"ops{qb}")
            for c in range(2):
                nc.tensor.matmul(
                    ops[:],
                    ctxn[c][:, qb * 128:(qb + 1) * 128],
                    w_sb["wo"][:, c * U:(c + 1) * U],
                    start=(c == 0), stop=False)
            nc.tensor.matmul(ops[:], ones_f[:], bo_sb[:],
                             start=False, stop=True, skip_group_check=True)
            nc.vector.tensor_copy(out_sb[:, qb * U:(qb + 1) * U], ops[:])

    prev = None
    prev_tiles = None
    for qc in range(QC):
        for g in range(2):
            cps[qc, g] = cpool.tile([128, QW], f32, tag="c", name=f"cps{qc}_{g}")
            zps[qc, g] = zpool.tile([128, QW], f32, tag="z", name=f"zps{qc}_{g}")
        for kb in range(NB):
            tiles = emit_scores(qc, kb)
            if prev is not None:
                emit_expmask_pvz(prev[0], prev[1], prev_tiles)
                if prev[1] == NB - 1:
                    emit_finish_qc(prev[0])
            prev, prev_tiles = (qc, kb), tiles
    emit_expmask_pvz(prev[0], prev[1], prev_tiles)
    emit_finish_qc(prev[0])

    nc.sync.dma_start(out_d.rearrange("(nb p) d -> p nb d", p=128),
                      out_sb.rearrange("p (nb d) -> p nb d", nb=NB))


_CACHED = None


def _get_program():
    global _CACHED
    if _CACHED is None:
        _CACHED = build_program()
    return _CACHED


def _bf16(a):
    return np.asarray(a, dtype=ml_dtypes.bfloat16)


def kernel(node_features, adjacency_matrix, Wq, Wk, Wv, Wo, bo, **run_kwargs):
    nc = _get_program()
    xT = _bf16(np.transpose(np.asarray(node_features, np.float32), (0, 2, 1)))
    adjT = _bf16(np.transpose(np.asarray(adjacency_matrix), (0, 2, 1)))
    diag = np.zeros((128, 128), dtype=ml_dtypes.bfloat16)
    np.fill_diagonal(diag, ml_dtypes.bfloat16(MASK_GAIN))
    wq, wk, wv, wo = _bf16(Wq), _bf16(Wk), _bf16(Wv), _bf16(Wo)
    bo32 = np.asarray(bo, np.float32)
    in_maps = []
    for b in range(B):
        in_maps.append({
            "xT": np.ascontiguousarray(xT[b]),
            "adjT": np.ascontiguousarray(adjT[b]),
            "Wq": wq, "Wk": wk, "Wv": wv, "Wo": wo,
            "bo": bo32, "diag": diag,
        })
    res = run_bass_kernel_spmd(nc, in_maps, core_ids=list(range(B)), **run_kwargs)
    out = np.stack([res.results[b]["out"] for b in range(B)], axis=0)
    kernel.last_results = res
    return out


# revision 14
# speedup vs baseline: 1.0829x; 1.0829x over previous
"""MultiHeadGraphAttention Trainium2 kernel (pipelined v2).

Data-parallel over batch: core b computes batch element b (B=8, 8 cores).

Per-core math (one batch element, N=2048 nodes, U=256 units, H=8 heads, d=32):
  Q = x Wq, K = x Wk, V = x Wv
  sT[k,q]  = sum_d KT[d,k] QT[d,q]           (scores, transposed layout)
  e        = exp(sT/sqrt(d)) * adjT          (masked exp)
  ctxT[d,q] = sum_k V[k,d] e[k,q] ; Z[q] = sum_k e[k,q]
  out      = (ctxT/Z).T @ Wo + bo

Structure: the [N,N] score plane is processed in (qc, kb) blocks of
[128 keys x 512 queries], 4 head-PAIR PSUM tiles of [128, 2x512] each
(2 banks, double-buffered) so ACT exp / DVE mask / PE matmuls pipeline
across pairs and iterations.  Per-pair mode:
  'A': ACT exp (PSUM->SBUF bf16) then DVE mask-multiply (bf16 2x)
  'P': PE adds 170*mask into PSUM (diag matmul), ACT does exp(scale*s-30)
       -> masked entries become exp(s-30) ~= 1e-13; no DVE pass at all
  'S': DVE-only Schraudolph: i16 = A*s + B (1x from PSUM), bitcast to
       bf16 ~= exp(scale*s), then mask-multiply (2x)
Host pre-transposes x/adj and converts everything to bf16.
"""

import sys

for p in ("/opt/trn_rl_repo",):
    if p not in sys.path:
        sys.path.insert(0, p)

from contextlib import ExitStack

import numpy as np
import ml_dtypes

import concourse.bass as bass
import concourse.mybir as mybir
import concourse.tile as tile
from concourse import bacc
from concourse.bass_utils import run_bass_kernel_spmd

B, N, U, H, D = 8, 2048, 256, 8, 32
NB = N // 128          # 16 key blocks of 128
QC = 4                 # q chunks
QW = N // QC           # 512 q per chunk
SCALE = 1.0 / np.sqrt(np.float32(D))
MASK_GAIN = 170.0      # bf16-exact; PE adds 170*m to raw scores ('P' mode)
MASK_BIAS = -(MASK_GAIN * float(SCALE))   # ACT bias cancels it where m=1
# Schraudolph bf16: bits(bf16(2^x)) ~= round(128*(x+127-0.04368))
SCH_A = 128.0 * float(SCALE) * 1.4426950408889634
SCH_B = (127.0 - 0.04368) * 128.0

f32 = mybir.dt.float32
bf16 = mybir.dt.bfloat16
i16 = mybir.dt.int16
EXP = mybir.ActivationFunctionType.Exp
MULT = mybir.AluOpType.mult
ADD = mybir.AluOpType.add


def pair_mode(qc, kb, p):
    """Which engine path computes masked-exp for head pair p of (qc,kb).

    Rotating split: every second (qc,kb) sends one pair (rotating across
    pairs so the approx error spreads over heads) to the DVE-Schraudolph
    path; everything else is ACT-exp + DVE-mask.  The 'P' path (PE adds
    170*mask into PSUM) measured slower: its K=128 matmuls serialize the
    PE array.
    """
    return "S" if (kb % 2 == 0 and p == (kb // 2) % 4) else "A"


def build_program():
    nc = bacc.Bacc("TRN2", target_bir_lowering=False, debug=False,
                   enable_asserts=False, num_devices=B)

    xT_d = nc.dram_tensor("xT", [U, N], bf16, kind="ExternalInput").ap()
    adjT_d = nc.dram_tensor("adjT", [N, N], bf16, kind="ExternalInput").ap()
    wq_d = nc.dram_tensor("Wq", [U, U], bf16, kind="ExternalInput").ap()
    wk_d = nc.dram_tensor("Wk", [U, U], bf16, kind="ExternalInput").ap()
    wv_d = nc.dram_tensor("Wv", [U, U], bf16, kind="ExternalInput").ap()
    wo_d = nc.dram_tensor("Wo", [U, U], bf16, kind="ExternalInput").ap()
    bo_d = nc.dram_tensor("bo", [U], f32, kind="ExternalInput").ap()
    diag_d = nc.dram_tensor("diag", [128, 128], bf16, kind="ExternalInput").ap()
    out_d = nc.dram_tensor("out", [N, U], f32, kind="ExternalOutput").ap()

    with tile.TileContext(nc) as tc:
        with ExitStack() as ctx:
            kernel_body(ctx, tc, xT_d, adjT_d, wq_d, wk_d, wv_d, wo_d,
                        bo_d, diag_d, out_d)
    nc.compile()
    return nc


def kernel_body(ctx, tc, xT_d, adjT_d, wq_d, wk_d, wv_d, wo_d, bo_d,
                diag_d, out_d):
    nc = tc.nc
    persist = ctx.enter_context(tc.tile_pool(name="persist", bufs=1))
    stage = ctx.enter_context(tc.tile_pool(name="stage", bufs=2))
    epool = ctx.enter_context(tc.tile_pool(name="epool", bufs=6))
    espool = ctx.enter_context(tc.tile_pool(name="espool", bufs=2))
    spool = ctx.enter_context(tc.tile_pool(name="spool", bufs=2, space="PSUM"))
    cpool = ctx.enter_context(tc.tile_pool(name="cpool", bufs=2, space="PSUM"))
    zpool = ctx.enter_context(tc.tile_pool(name="zpool", bufs=2, space="PSUM"))

    # ---- persistent SBUF tensors -------------------------------------------
    # mask, bf16 0/1, block kb at cols kb*N (loaded first: feeds everything)
    m_sb = persist.tile([128, NB * N], bf16, tag="m")
    for kb in range(NB):
        nc.sync.dma_start(m_sb[:, kb * N:(kb + 1) * N],
                          adjT_d[kb * 128:(kb + 1) * 128, :])
    # projections, transposed: chunk g holds heads 4g..4g+3 (head h at
    # partitions 32*(h%4) .. +32)
    qT = [persist.tile([128, N], bf16, tag=f"qT{c}", name=f"qT{c}") for c in range(2)]
    kT = [persist.tile([128, N], bf16, tag=f"kT{c}", name=f"kT{c}") for c in range(2)]
    v_sb = persist.tile([128, NB * U], bf16, tag="v")
    # weights: [128, 2*256] bf16, feature chunk c at cols c*U
    w_sb = {}
    for nm, dram in (("wq", wq_d), ("wk", wk_d), ("wv", wv_d), ("wo", wo_d)):
        w_sb[nm] = persist.tile([128, 2 * U], bf16, tag=nm, name=nm)
        for c in range(2):
            nc.sync.dma_start(w_sb[nm][:, c * U:(c + 1) * U],
                              dram[c * 128:(c + 1) * 128, :])
    bo_sb = persist.tile([1, U], f32, tag="bo")
    nc.sync.dma_start(bo_sb[:], bo_d.rearrange("(o n) -> o n", o=1))
    diag_sb = persist.tile([128, 128], bf16, tag="diag")
    nc.sync.dma_start(diag_sb[:], diag_d)
    ones_bf = persist.tile([128, 1], bf16, tag="ones_bf")
    nc.vector.memset(ones_bf[:], 1.0)
    mbias = persist.tile([128, 1], f32, tag="mbias")
    nc.vector.memset(mbias[:], float(MASK_BIAS))
    ones_f = persist.tile([1, 128], f32, tag="ones_f")
    nc.vector.memset(ones_f[:], 1.0)
    # normalized context (transposed) bf16; final output rows f32
    ctxn = [persist.tile([128, N], bf16, tag=f"ctxn{c}", name=f"ctxn{c}") for c in range(2)]
    out_sb = persist.tile([128, NB * U], f32, tag="out_sb")
    # per-qc denominator scratch (full tiles; rows 32j hold head (g,j))
    zrec = [persist.tile([128, QW], f32, tag=f"zrec{g}", name=f"zrec{g}") for g in range(2)]
    zbs = [persist.tile([128, QW], f32, tag=f"zbs{g}", name=f"zbs{g}") for g in range(2)]
    zinv = [persist.tile([128, QW], f32, tag=f"zinv{g}", name=f"zinv{g}") for g in range(2)]

    # ---- projections (bf16 operands) ---------------------------------------
    xT = [stage.tile([128, N], bf16, tag="stage", name=f"xT{c}") for c in range(2)]
    for c in range(2):
        nc.sync.dma_start(xT[c][:], xT_d[c * 128:(c + 1) * 128, :])

    # Q/K: per dst chunk g, produce [128, N] via 4 x [128,512] PSUM pieces
    for w, dst in (("wq", qT), ("wk", kT)):
        for g in range(2):
            for nn in range(2):       # 1024-wide double-slices of N tokens
                ps = spool.tile([128, 2 * QW], f32, tag="s", name="projps")
                for half in range(2):
                    sl = slice(half * QW, (half + 1) * QW)
                    tok = slice(nn * 2 * QW + half * QW,
                                nn * 2 * QW + (half + 1) * QW)
                    for kc in range(2):
                        nc.tensor.matmul(
                            ps[:, sl],
                            w_sb[w][:, (kc * 2 + g) * 128:(kc * 2 + g + 1) * 128],
                            xT[kc][:, tok],
                            start=(kc == 0), stop=(kc == 1))
                nc.scalar.copy(dst[g][:, nn * 2 * QW:(nn + 1) * 2 * QW], ps[:])
    # V = x @ Wv, natural layout, bf16
    for kb in range(NB):
        ps = cpool.tile([128, U], f32, tag="c", name="vps")
        for kc in range(2):
            nc.tensor.matmul(
                ps[:],
                xT[kc][:, kb * 128:(kb + 1) * 128],
                w_sb["wv"][:, kc * U:(kc + 1) * U],
                start=(kc == 0), stop=(kc == 1))
        nc.vector.tensor_copy(v_sb[:, kb * U:(kb + 1) * U], ps[:])

    # ---- main attention loop, one-step PE software pipeline -----------------
    # pair p covers heads (2p, 2p+1): g = p//2, row/col groups j0=2*(p%2), j0+1
    cps = {}
    zps = {}

    def emit_scores(qc, kb):
        qs = qc * QW
        tiles = []
        for p in range(4):
            g, j0 = p // 2, 2 * (p % 2)
            mode = pair_mode(qc, kb, p)
            sps = spool.tile([128, 2 * QW], f32, tag="s", name=f"sps{qc}_{kb}_{p}")
            for jj in range(2):
                j = j0 + jj
                nc.tensor.matmul(
                    sps[:, jj * QW:(jj + 1) * QW],
                    kT[g][32 * j:32 * (j + 1), kb * 128:(kb + 1) * 128],
                    qT[g][32 * j:32 * (j + 1), qs:qs + QW],
                    start=True, stop=(mode != "P"),
                    tile_position=(32 * j, 0))
            if mode == "P":
                # accumulate 170*m into both head slices, full-array matmul
                me = m_sb[:, kb * N + qs:kb * N + qs + QW]
                for jj in range(2):
                    nc.tensor.matmul(
                        sps[:, jj * QW:(jj + 1) * QW],
                        diag_sb[:], me,
                        start=False, stop=True, skip_group_check=True)
            tiles.append((sps, mode))
        return tiles

    def emit_expmask_pvz(qc, kb, tiles):
        qs = qc * QW
        for p in range(4):
            g, j0 = p // 2, 2 * (p % 2)
            sps, mode = tiles[p]
            e = epool.tile([128, 2 * QW], bf16, tag="e", name=f"e{qc}_{kb}_{p}")
            me = m_sb[:, kb * N + qs:kb * N + qs + QW]
            if mode == "A":
                nc.scalar.activation(e[:], sps[:], EXP, scale=float(SCALE))
                nc.vector.tensor_tensor(
                    e.rearrange("p (j q) -> p j q", j=2),
                    e.rearrange("p (j q) -> p j q", j=2),
                    me.unsqueeze(1).broadcast_to([128, 2, QW]), MULT)
            elif mode == "P":
                nc.scalar.activation(e[:], sps[:], EXP, scale=float(SCALE),
                                     bias=mbias[:])
            else:  # "S"
                es = espool.tile([128, 2 * QW], i16, tag="es",
                                 name=f"es{qc}_{kb}_{p}")
                nc.vector.tensor_scalar(es[:], sps[:], float(SCH_A),
                                        float(SCH_B), MULT, ADD)
                nc.vector.tensor_tensor(
                    e.rearrange("p (j q) -> p j q", j=2),
                    es.bitcast(bf16).rearrange("p (j q) -> p j q", j=2),
                    me.unsqueeze(1).broadcast_to([128, 2, QW]), MULT)
            for jj in range(2):
                j = j0 + jj
                ej = e[:, jj * QW:(jj + 1) * QW]
                h = 4 * g + j
                nc.tensor.matmul(
                    cps[qc, g][32 * j:32 * (j + 1), :],
                    v_sb[:, kb * U + h * D:kb * U + (h + 1) * D],
                    ej, start=(kb == 0), stop=(kb == NB - 1),
                    tile_position=(0, 32 * j))
                # Z row lives at the OPPOSITE col-group pair so it can run
                # concurrently with the PV matmul of the same head
                jz = (j + 2) % 4
                nc.tensor.matmul(
                    zps[qc, g][32 * jz:32 * jz + 1, :],
                    ones_bf[:], ej,
                    start=(kb == 0), stop=(kb == NB - 1),
                    tile_position=(0, 32 * jz))

    def emit_finish_qc(qc):
        qs = qc * QW
        for g in range(2):
            nc.vector.tensor_copy(zrec[g][:], zps[qc, g][:])
            for j in range(4):
                jz = (j + 2) % 4   # Z of head (g,j) lives at row 32*jz
                nc.sync.dma_start(
                    zbs[g][32 * j:32 * (j + 1), :],
                    zrec[g][32 * jz:32 * jz + 1, :]
                    .unsqueeze(1).broadcast_to([1, 32, QW]))
            nc.vector.reciprocal_approx_fast(zinv[g][:], zbs[g][:])
            nc.vector.tensor_tensor(ctxn[g][:, qs:qs + QW], cps[qc, g][:],
                                    zinv[g][:], MULT)
        for qb in range(qc * QC, (qc + 1) * QC):
            ops = cpool.tile([128, U], f32, tag="c", name=f"ops{qb}")
            for c in range(2):
                nc.tensor.matmul(
                    ops[:],
                    ctxn[c][:, qb * 128:(qb + 1) * 128],
                    w_sb["wo"][:, c * U:(c + 1) * U],
                    start=(c == 0), stop=False)
            nc.tensor.matmul(ops[:], ones_f[:], bo_sb[:],
                             start=False, stop=True, skip_group_check=True)
            nc.vector.tensor_copy(out_sb[:, qb * U:(qb + 1) * U], ops[:])
            nc.sync.dma_start(
                out_d[qb * 128:(qb + 1) * 128, :],
                out_sb[:, qb * U:(qb + 1) * U])

    # flat schedule with one-step PE stagger
    prev = None
    prev_tiles = None
    for qc in range(QC):
        for g in range(2):
            cps[qc, g] = cpool.tile([128, QW], f32, tag="c", name=f"cps{qc}_{g}")
            zps[qc, g] = zpool.tile([128, QW], f32, tag="z", name=f"zps{qc}_{g}")
        for kb in range(NB):
            tiles = emit_scores(qc, kb)
            if prev is not None:
                emit_expmask_pvz(prev[0], prev[1], prev_tiles)
                if prev[1] == NB - 1:
                    emit_finish_qc(prev[0])
            prev, prev_tiles = (qc, kb), tiles
    emit_expmask_pvz(prev[0], prev[1], prev_tiles)
    emit_finish_qc(prev[0])


_CACHED = None


def _get_program():
    global _CACHED
    if _CACHED is None:
        _CACHED = build_program()
    return _CACHED


def _bf16(a):
    return np.asarray(a, dtype=ml_dtypes.bfloat16)


def kernel(node_features, adjacency_matrix, Wq, Wk, Wv, Wo, bo, **run_kwargs):
    nc = _get_program()
    xT = _bf16(np.transpose(np.asarray(node_features, np.float32), (0, 2, 1)))
    adjT = _bf16(np.transpose(np.asarray(adjacency_matrix), (0, 2, 1)))
    diag = np.zeros((128, 128), dtype=ml_dtypes.bfloat16)
    np.fill_diagonal(diag, ml_dtypes.bfloat16(MASK_GAIN))
    wq, wk, wv, wo = _bf16(Wq), _bf16(Wk), _bf16(Wv), _bf16(Wo)
    bo32 = np.asarray(bo, np.float32)
    in_maps = []
    for b in range(B):
        in_maps.append({
            "xT": np.ascontiguousarray(xT[b]),
            "adjT": np.ascontiguousarray(adjT[b]),
            "Wq": wq, "Wk": wk, "Wv": wv, "Wo": wo,
            "bo": bo32, "diag": diag,
        })
    res = run_bass_kernel_spmd(nc, in_maps, core_ids=list(range(B)), **run_kwargs)
    out = np.stack([res.results[b]["out"] for b in range(B)], axis=0)
    kernel.last_results = res
    return out
